# revision 37
# baseline (speedup 1.0000x reference)
"""Trainium2 Bass kernel for nn_EntityAggregator (segment_reduce).

Reference computation per (b, t):
    query_inst   = query_feat @ W_inst + b_inst          # (64, 32)
    query_motion = query_feat @ W_motion + b_motion      # (64, 16)
    bg   = assignment_prob[..., 0]                       # (pix,)
    fg   = assignment_prob[..., 1:65]                    # (pix, 64)
    norm = fg.sum(-1)                                    # (pix,)
    pooled = fg @ [query_inst | query_motion]            # (pix, 48)
    scale  = (1 - bg) / norm
    inst   = instance_affinity + scale[:, None] * pooled[:, :32]
    motion = motion_code       + scale[:, None] * pooled[:, 32:48]
    entity_id = assigned_query (pass-through)

Sharding: data-parallel over the 8 (b, t) pairs -> 8 NeuronCores, one
(b, t) shard each.  pix = V*H*W = 65536 per core.

Per-core device strategy (pixel p_idx = p*NJ + j, p = partition):
  - all loads/stores are per-partition-contiguous DMAs
  - fg must be contracted over q (free dim), so each pair of j columns
    [128, 2*64] is PE-transposed to [128(q2), 128(p)], copied to SBUF,
    and matmul'd against a fixed block-diagonal stationary
    Sfix[(j2,c),(j2,d)] = Qext[c,d] (Qext = [Q48 | ones]) giving
    pooled (+ fg row-sum) back in pixel-major layout [128, 2, 49].
  - epilogue on DVE: recip of norm, scale = (1-bg)*recip, broadcast
    multiply, add instance/motion, store.
"""

import ml_dtypes
import numpy as np

import concourse.bass as bass
import concourse.mybir as mybir
import concourse.tile as tile
from concourse.bass import MemorySpace
from concourse.masks import make_identity

F32 = mybir.dt.float32
BF16 = mybir.dt.bfloat16

# bf16 PE path: fg is host-cast to bf16 (halves the dominant DMA), the
# transpose + pooling matmul run in bf16 with fp32 PSUM accumulation.
# Measured end-to-end rel-to-scale error ~1.1e-4 (vs 6.4e-6 for the f32r
# path) — flip to False to revert to float32r compute on f32 fg.
USE_BF16 = True

# problem constants (per core shard = one (b, t) pair)
NPIX = 4 * 128 * 128  # v*h*w pixels per core
NQ = 64
QDIM = 256
IDIM = 32
MDIM = 16
DCAT = IDIM + MDIM  # 48
NCOL = NQ + 1  # 65 assignment-prob columns
P = 128  # partitions


def _split_multiwaits(nc):
    """walrus in this toolchain allows only ONE embedded sync-wait per
    instruction (setupSyncWait: 'Too many sync wait commands').  Tile
    emits several on phase-boundary instructions.  Hoist all-but-one
    wait onto same-engine NoOps inserted right before the offender —
    semantics preserved (same engine, program order)."""
    n_split = 0
    for f in nc.m.functions:
        for bb in f.blocks:
            insts = bb.instructions
            new = []
            k = 0
            for ins in insts:
                si = getattr(ins, "sync_info", None)
                ow = list(si.on_wait) if si is not None and si.on_wait else []
                if len(ow) > 1:
                    for w in ow[:-1]:
                        nop = mybir.InstNoOp(name=f"{ins.name}-w{k}")
                        k += 1
                        nop.engine = ins.engine
                        nop.sync_info = mybir.SyncInfo(on_wait=[w], on_update=[])
                        new.append(nop)
                        n_split += 1
                    ins.sync_info = mybir.SyncInfo(
                        on_wait=[ow[-1]],
                        on_update=list(si.on_update) if si.on_update else [],
                    )
                new.append(ins)
            insts[:] = new
    return n_split


def _build(npix=NPIX, jt=64, gj=8, split=True):
    """Build the per-core Bass program.

    npix: pixels for this core (must be divisible by 128*jt)
    jt:   j-columns per DMA tile (pixels-per-partition per tile)
    gj:   j-columns per PSUM group (gj*49 must fit a 2KB PSUM bank)
    """
    nj = npix // P  # pixels per partition
    assert nj % jt == 0
    assert jt % gj == 0
    assert gj % 2 == 0
    assert (jt // gj) % 2 == 0  # epilogue processes group pairs
    assert gj * (DCAT + 1) * 4 <= 2048  # one PSUM bank
    ntiles = nj // jt
    ngrp = jt // gj
    nunit = gj // 2

    nc = bass.Bass()

    fg_dt = BF16 if USE_BF16 else F32
    fg_d = nc.dram_tensor("fg", [npix, NQ], fg_dt, kind="ExternalInput")
    bg_d = nc.dram_tensor("bg", [npix], F32, kind="ExternalInput")
    ia_d = nc.dram_tensor("ia", [npix, IDIM], F32, kind="ExternalInput")
    mc_d = nc.dram_tensor("mc", [npix, MDIM], F32, kind="ExternalInput")
    qf_d = nc.dram_tensor("qf", [NQ, QDIM], F32, kind="ExternalInput")
    wi_d = nc.dram_tensor("wi", [QDIM, IDIM], F32, kind="ExternalInput")
    bi_d = nc.dram_tensor("bi", [IDIM], F32, kind="ExternalInput")
    wm_d = nc.dram_tensor("wm", [QDIM, MDIM], F32, kind="ExternalInput")
    bm_d = nc.dram_tensor("bm", [MDIM], F32, kind="ExternalInput")
    oi_d = nc.dram_tensor("oi", [npix, IDIM], F32, kind="ExternalOutput")
    om_d = nc.dram_tensor("om", [npix, MDIM], F32, kind="ExternalOutput")

    fgv = fg_d[:].rearrange("(p j) c -> p j c", p=P)
    bgv = bg_d[:].rearrange("(p j) -> p j", p=P)
    iav = ia_d[:].rearrange("(p j) c -> p j c", p=P)
    mcv = mc_d[:].rearrange("(p j) c -> p j c", p=P)
    oiv = oi_d[:].rearrange("(p j) c -> p j c", p=P)
    omv = om_d[:].rearrange("(p j) c -> p j c", p=P)

    with tile.TileContext(nc) as tc:
        with tc.tile_pool(name="consts", bufs=1) as consts:
            ident = consts.tile([P, P], F32)
            make_identity(nc, ident)
            if USE_BF16:
                ident_c = consts.tile([P, P], BF16)
                nc.vector.tensor_copy(ident_c, ident)
            else:
                ident_c = ident

            # ---- per-core setup: Q48 = qf @ [W_inst | W_motion] + bias ----
            qf_sb = consts.tile([NQ, QDIM], F32)
            nc.sync.dma_start(out=qf_sb, in_=qf_d[:])
            wcat = consts.tile([P, 2, DCAT], F32)
            nc.sync.dma_start(out=wcat[:, 0, 0:IDIM], in_=wi_d[0:P, :])
            nc.sync.dma_start(out=wcat[:, 1, 0:IDIM], in_=wi_d[P : 2 * P, :])
            nc.sync.dma_start(out=wcat[:, 0, IDIM:DCAT], in_=wm_d[0:P, :])
            nc.sync.dma_start(out=wcat[:, 1, IDIM:DCAT], in_=wm_d[P : 2 * P, :])
            bcat = consts.tile([NQ, DCAT], F32)
            nc.sync.dma_start(
                out=bcat[:, 0:IDIM],
                in_=bass.AP(tensor=bi_d, offset=0, ap=[[0, NQ], [1, IDIM]]),
            )
            nc.sync.dma_start(
                out=bcat[:, IDIM:DCAT],
                in_=bass.AP(tensor=bm_d, offset=0, ap=[[0, NQ], [1, MDIM]]),
            )

            qfT_sb = consts.tile([P, 2, NQ], F32)
            q48_sb = consts.tile([NQ, DCAT], F32)
            sfix = consts.tile([P, 2, DCAT + 1], F32)

            with tc.tile_pool(name="setup_ps", bufs=1, space=MemorySpace.PSUM) as sps:
                qfT_ps = sps.tile([P, 2, NQ], F32)
                for cc in range(2):
                    nc.tensor.transpose(
                        qfT_ps[:, cc, :],
                        qf_sb[:, cc * P : (cc + 1) * P],
                        ident[0:NQ, 0:NQ],
                    )
                nc.scalar.copy(qfT_sb, qfT_ps)
                q48_ps = sps.tile([NQ, DCAT], F32)
                nc.tensor.matmul(
                    q48_ps, qfT_sb[:, 0, :], wcat[:, 0, :], start=True, stop=False
                )
                nc.tensor.matmul(
                    q48_ps, qfT_sb[:, 1, :], wcat[:, 1, :], start=False, stop=True
                )
                nc.vector.tensor_add(q48_sb, q48_ps, bcat)

            # Sfix block-diagonal: [(j2,c), (j2,d)] = Qext[c,d]; Qext=[Q48|1]
            nc.vector.memset(sfix, 0.0)
            nc.vector.memset(sfix[0:NQ, 0, DCAT : DCAT + 1], 1.0)
            nc.vector.memset(sfix[NQ : 2 * NQ, 1, DCAT : DCAT + 1], 1.0)
            nc.vector.tensor_copy(sfix[0:NQ, 0, 0:DCAT], q48_sb)
            # partition-shifted copy must go through DMA
            nc.sync.dma_start(out=sfix[NQ : 2 * NQ, 1, 0:DCAT], in_=q48_sb)
            # compute-dtype copy for the pooling matmul: bf16 (FWL weight
            # load) or fp32r (single-pass fp32; plain f32 is 2-pass LO/HI)
            cdt = BF16 if USE_BF16 else mybir.dt.float32r
            sfix_r = consts.tile([P, 2, DCAT + 1], cdt)
            nc.vector.tensor_copy(sfix_r, sfix)

            # ---- main loop ----
            with (
                tc.tile_pool(name="ap_pool", bufs=3) as ap_pool,
                tc.tile_pool(name="ia_pool", bufs=3) as ia_pool,
                tc.tile_pool(name="mc_pool", bufs=3) as mc_pool,
                tc.tile_pool(name="out_pool", bufs=2) as out_pool,
                tc.tile_pool(name="gate_pool", bufs=2) as gate_pool,
                tc.tile_pool(name="aptr_pool", bufs=2) as aptr_pool,
                tc.tile_pool(name="small_pool", bufs=6) as small_pool,
                tc.tile_pool(name="scaled_pool", bufs=4) as scaled_pool,
                tc.tile_pool(name="psT", bufs=2, space=MemorySpace.PSUM) as psT,
                tc.tile_pool(name="psG", bufs=3, space=MemorySpace.PSUM) as psG,
            ):
                def emit_loads(it):
                    j0 = it * jt
                    # fg is host-repacked c-contiguous: full-rate DMA and
                    # each 2-j unit is one contiguous 128-col slice (matmul
                    # stationary operand requires a single free dimension)
                    fg_t = ap_pool.tile([P, jt, NQ], fg_dt, tag="fg")
                    nc.sync.dma_start(out=fg_t, in_=fgv[:, j0 : j0 + jt, :])
                    bg_t = ap_pool.tile([P, jt], F32, tag="bg")
                    nc.sync.dma_start(out=bg_t, in_=bgv[:, j0 : j0 + jt])
                    ia_t = ia_pool.tile([P, jt, IDIM], F32, tag="ia")
                    nc.sync.dma_start(out=ia_t, in_=iav[:, j0 : j0 + jt, :])
                    mc_t = mc_pool.tile([P, jt, MDIM], F32, tag="mc")
                    nc.sync.dma_start(out=mc_t, in_=mcv[:, j0 : j0 + jt, :])
                    return fg_t, bg_t, ia_t, mc_t

                loads_q = [emit_loads(0)]
                if ntiles > 1:
                    loads_q.append(emit_loads(1))
                for it in range(ntiles):
                    j0 = it * jt
                    fg_t, bg_t, ia_t, mc_t = loads_q.pop(0)
                    # prefetch loads two tiles ahead: slots (bufs=3) are
                    # long-free so the DMAs issue and transfer early
                    if it + 2 < ntiles:
                        loads_q.append(emit_loads(it + 2))

                    # gate = 1 - bg  (bg = prob column 0 of each j)
                    gate_t = gate_pool.tile([P, jt], F32)
                    nc.vector.tensor_scalar(
                        out=gate_t,
                        in0=bg_t,
                        scalar1=-1.0,
                        scalar2=1.0,
                        op0=mybir.AluOpType.mult,
                        op1=mybir.AluOpType.add,
                    )

                    oi_t = out_pool.tile([P, jt, IDIM], F32, tag="oi")
                    om_t = out_pool.tile([P, jt, MDIM], F32, tag="om")

                    # One t_pair = 2 groups = 2*nunit 2-j transposes packed
                    # into ONE psum bank (bf16) + ONE ACT copy.  Transposes
                    # for pair p+1 are emitted before pair p's matmuls so
                    # the in-order PE never stalls on the ACT copy.
                    def t_pair(p):
                        gl = p * 2 * gj
                        nu2 = 2 * nunit
                        apT_ps = psT.tile([P, nu2, P], fg_dt, tag="apT_ps")
                        for u in range(nu2):
                            jl = gl + u * 2
                            nc.tensor.transpose(
                                apT_ps[:, u, :],
                                fg_t[:, jl : jl + 2, :].rearrange(
                                    "p j c -> p (j c)"
                                ),
                                ident_c,
                            )
                        # single psum->sbuf copy (casts to the compute dtype)
                        apT_sb = aptr_pool.tile([P, nu2, P], cdt, tag="apT_sb")
                        nc.scalar.copy(apT_sb, apT_ps)
                        return apT_sb

                    # epilogue processes PAIRS of groups from one 2-bank
                    # psum tile [P, 2, 512] to halve DVE op count
                    npair = ngrp // 2
                    BANKF = 512  # f32 elems per psum bank
                    apT_cur = t_pair(0)
                    for pr in range(npair):
                        apT_nxt = t_pair(pr + 1) if pr + 1 < npair else None
                        pg2 = psG.tile([P, 2, BANKF], F32)
                        for h in range(2):
                            for u in range(nunit):
                                nc.tensor.matmul(
                                    pg2[:, h, u * 98 : u * 98 + 98],
                                    apT_cur[:, h * nunit + u, :],
                                    sfix_r,
                                    start=True,
                                    stop=True,
                                )
                        apT_cur = apT_nxt
                        gl = pr * 2 * gj  # local j base of the pair (16 j)
                        jj = 2 * gj
                        # [P, 2, gj, 49] view of the pair's pooled+norm cols
                        pgv = pg2[:, :, 0 : gj * (DCAT + 1)].rearrange(
                            "p h (j k) -> p h j k", k=DCAT + 1
                        )
                        recip_t = small_pool.tile([P, 2, gj], F32, tag="recip")
                        nc.vector.reciprocal(recip_t, pgv[:, :, :, DCAT])
                        scale_t = small_pool.tile([P, 2, gj], F32, tag="scale")
                        nc.vector.tensor_mul(
                            scale_t,
                            recip_t,
                            gate_t[:, gl : gl + jj].rearrange(
                                "p (h j) -> p h j", h=2
                            ),
                        )
                        scale_b = bass.AP(
                            tensor=scale_t.tensor,
                            offset=scale_t.offset,
                            ap=[
                                scale_t.ap[0],
                                scale_t.ap[1],
                                scale_t.ap[2],
                                [0, DCAT],
                            ],
                        )
                        scaled = scaled_pool.tile([P, 2, gj, DCAT], F32)
                        nc.vector.tensor_mul(
                            scaled, pgv[:, :, :, 0:DCAT], scale_b
                        )
                        # adds split across GpSimd (inst) and DVE (motion)
                        nc.gpsimd.tensor_add(
                            oi_t[:, gl : gl + jj, :].rearrange(
                                "p (h j) c -> p h j c", h=2
                            ),
                            scaled[:, :, :, 0:IDIM],
                            ia_t[:, gl : gl + jj, :].rearrange(
                                "p (h j) c -> p h j c", h=2
                            ),
                        )
                        nc.vector.tensor_add(
                            om_t[:, gl : gl + jj, :].rearrange(
                                "p (h j) c -> p h j c", h=2
                            ),
                            scaled[:, :, :, IDIM:DCAT],
                            mc_t[:, gl : gl + jj, :].rearrange(
                                "p (h j) c -> p h j c", h=2
                            ),
                        )

                    nc.sync.dma_start(out=oiv[:, j0 : j0 + jt, :], in_=oi_t)
                    nc.sync.dma_start(out=omv[:, j0 : j0 + jt, :], in_=om_t)

    if split:
        _split_multiwaits(nc)
    return nc


_NC_CACHE = {}


def _get_nc(npix=NPIX, jt=64, gj=8):
    key = (npix, jt, gj)
    if key not in _NC_CACHE:
        _NC_CACHE[key] = _build(npix, jt, gj)
    return _NC_CACHE[key]


def _make_in_maps(inputs):
    ap = np.ascontiguousarray(np.asarray(inputs["assignment_prob"], np.float32))
    ia = np.ascontiguousarray(np.asarray(inputs["instance_affinity"], np.float32))
    mc = np.ascontiguousarray(np.asarray(inputs["motion_code"], np.float32))
    qf = np.ascontiguousarray(np.asarray(inputs["query_feat"], np.float32))
    wi = np.ascontiguousarray(np.asarray(inputs["W_inst"], np.float32))
    bi = np.ascontiguousarray(np.asarray(inputs["b_inst"], np.float32))
    wm = np.ascontiguousarray(np.asarray(inputs["W_motion"], np.float32))
    bm = np.ascontiguousarray(np.asarray(inputs["b_motion"], np.float32))
    B, T = ap.shape[0], ap.shape[1]
    in_maps = []
    for b in range(B):
        for t in range(T):
            apc = ap[b, t].reshape(NPIX, NCOL)
            in_maps.append(
                {
                    "fg": (
                        np.ascontiguousarray(apc[:, 1:]).astype(
                            ml_dtypes.bfloat16
                        )
                        if USE_BF16
                        else np.ascontiguousarray(apc[:, 1:])
                    ),
                    "bg": np.ascontiguousarray(apc[:, 0]),
                    "ia": np.ascontiguousarray(ia[b, t].reshape(NPIX, IDIM)),
                    "mc": np.ascontiguousarray(mc[b, t].reshape(NPIX, MDIM)),
                    "qf": np.ascontiguousarray(qf[b, t]),
                    "wi": wi,
                    "bi": bi,
                    "wm": wm,
                    "bm": bm,
                }
            )
    return in_maps


def _run(inputs, trace=False, tmpdir=None):
    from concourse.bass_utils import run_bass_kernel_spmd

    nc = _get_nc()
    in_maps = _make_in_maps(inputs)
    res = run_bass_kernel_spmd(
        nc, in_maps, list(range(8)), trace=trace, tmpdir=tmpdir
    )
    ia = np.asarray(inputs["instance_affinity"])
    B, T, V, H, W_ = ia.shape[:5]
    inst = np.stack([res.results[k]["oi"] for k in range(8)]).reshape(
        B, T, V, H, W_, IDIM
    )
    mot = np.stack([res.results[k]["om"] for k in range(8)]).reshape(
        B, T, V, H, W_, MDIM
    )
    aq = np.asarray(inputs["assigned_query"])
    return (aq, inst, mot), res


def kernel(**inputs):
    out, _ = _run(inputs, trace=False)
    return out


# revision 40
# speedup vs baseline: 1.0979x; 1.0979x over previous
"""Trainium2 Bass kernel for nn_EntityAggregator (segment_reduce).

Reference computation per (b, t):
    query_inst   = query_feat @ W_inst + b_inst          # (64, 32)
    query_motion = query_feat @ W_motion + b_motion      # (64, 16)
    bg   = assignment_prob[..., 0]                       # (pix,)
    fg   = assignment_prob[..., 1:65]                    # (pix, 64)
    norm = fg.sum(-1)                                    # (pix,)
    pooled = fg @ [query_inst | query_motion]            # (pix, 48)
    scale  = (1 - bg) / norm
    inst   = instance_affinity + scale[:, None] * pooled[:, :32]
    motion = motion_code       + scale[:, None] * pooled[:, 32:48]
    entity_id = assigned_query (pass-through)

Sharding: data-parallel over the 8 (b, t) pairs -> 8 NeuronCores, one
(b, t) shard each.  pix = V*H*W = 65536 per core.

Per-core device strategy (pixel p_idx = p*NJ + j, p = partition):
  - all loads/stores are per-partition-contiguous DMAs
  - fg must be contracted over q (free dim), so each pair of j columns
    [128, 2*64] is PE-transposed to [128(q2), 128(p)], copied to SBUF,
    and matmul'd against a fixed block-diagonal stationary
    Sfix[(j2,c),(j2,d)] = Qext[c,d] (Qext = [Q48 | ones]) giving
    pooled (+ fg row-sum) back in pixel-major layout [128, 2, 49].
  - epilogue on DVE: recip of norm, scale = (1-bg)*recip, broadcast
    multiply, add instance/motion, store.
"""

import ml_dtypes
import numpy as np

import concourse.bass as bass
import concourse.mybir as mybir
import concourse.tile as tile
from concourse.bass import MemorySpace
from concourse.masks import make_identity

F32 = mybir.dt.float32
BF16 = mybir.dt.bfloat16

# bf16 PE path: fg is host-cast to bf16 (halves the dominant DMA), the
# transpose + pooling matmul run in bf16 with fp32 PSUM accumulation.
# Measured end-to-end rel-to-scale error ~1.1e-4 (vs 6.4e-6 for the f32r
# path) — flip to False to revert to float32r compute on f32 fg.
USE_BF16 = True

# problem constants (per core shard = one (b, t) pair)
NPIX = 4 * 128 * 128  # v*h*w pixels per core
NQ = 64
QDIM = 256
IDIM = 32
MDIM = 16
DCAT = IDIM + MDIM  # 48
NCOL = NQ + 1  # 65 assignment-prob columns
P = 128  # partitions


def _split_multiwaits(nc):
    """walrus in this toolchain allows only ONE embedded sync-wait per
    instruction (setupSyncWait: 'Too many sync wait commands').  Tile
    emits several on phase-boundary instructions.  Hoist all-but-one
    wait onto same-engine NoOps inserted right before the offender —
    semantics preserved (same engine, program order)."""
    n_split = 0
    for f in nc.m.functions:
        for bb in f.blocks:
            insts = bb.instructions
            new = []
            k = 0
            for ins in insts:
                si = getattr(ins, "sync_info", None)
                ow = list(si.on_wait) if si is not None and si.on_wait else []
                if len(ow) > 1:
                    for w in ow[:-1]:
                        nop = mybir.InstNoOp(name=f"{ins.name}-w{k}")
                        k += 1
                        nop.engine = ins.engine
                        nop.sync_info = mybir.SyncInfo(on_wait=[w], on_update=[])
                        new.append(nop)
                        n_split += 1
                    ins.sync_info = mybir.SyncInfo(
                        on_wait=[ow[-1]],
                        on_update=list(si.on_update) if si.on_update else [],
                    )
                new.append(ins)
            insts[:] = new
    return n_split


def _build(npix=NPIX, jt=64, gj=8, split=True):
    """Build the per-core Bass program.

    npix: pixels for this core (must be divisible by 128*jt)
    jt:   j-columns per DMA tile (pixels-per-partition per tile)
    gj:   j-columns per PSUM group (gj*49 must fit a 2KB PSUM bank)
    """
    nj = npix // P  # pixels per partition
    assert nj % jt == 0
    assert jt % gj == 0
    assert gj % 2 == 0
    assert (jt // gj) % 2 == 0  # epilogue processes group pairs
    assert gj * (DCAT + 1) * 4 <= 2048  # one PSUM bank
    ntiles = nj // jt
    ngrp = jt // gj
    nunit = gj // 2

    nc = bass.Bass()

    fg_dt = BF16 if USE_BF16 else F32
    fg_d = nc.dram_tensor("fg", [npix, NQ], fg_dt, kind="ExternalInput")
    bg_d = nc.dram_tensor("bg", [npix], F32, kind="ExternalInput")
    ia_d = nc.dram_tensor("ia", [npix, IDIM], F32, kind="ExternalInput")
    mc_d = nc.dram_tensor("mc", [npix, MDIM], F32, kind="ExternalInput")
    qf_d = nc.dram_tensor("qf", [NQ, QDIM], F32, kind="ExternalInput")
    wi_d = nc.dram_tensor("wi", [QDIM, IDIM], F32, kind="ExternalInput")
    bi_d = nc.dram_tensor("bi", [IDIM], F32, kind="ExternalInput")
    wm_d = nc.dram_tensor("wm", [QDIM, MDIM], F32, kind="ExternalInput")
    bm_d = nc.dram_tensor("bm", [MDIM], F32, kind="ExternalInput")
    oi_d = nc.dram_tensor("oi", [npix, IDIM], F32, kind="ExternalOutput")
    om_d = nc.dram_tensor("om", [npix, MDIM], F32, kind="ExternalOutput")

    fgv = fg_d[:].rearrange("(p j) c -> p j c", p=P)
    bgv = bg_d[:].rearrange("(p j) -> p j", p=P)
    iav = ia_d[:].rearrange("(p j) c -> p j c", p=P)
    mcv = mc_d[:].rearrange("(p j) c -> p j c", p=P)
    oiv = oi_d[:].rearrange("(p j) c -> p j c", p=P)
    omv = om_d[:].rearrange("(p j) c -> p j c", p=P)

    with tile.TileContext(nc) as tc:
        with tc.tile_pool(name="consts", bufs=1) as consts:
            ident = consts.tile([P, P], F32)
            make_identity(nc, ident)
            if USE_BF16:
                ident_c = consts.tile([P, P], BF16)
                nc.vector.tensor_copy(ident_c, ident)
            else:
                ident_c = ident

            # ---- per-core setup: Q48 = qf @ [W_inst | W_motion] + bias ----
            qf_sb = consts.tile([NQ, QDIM], F32)
            nc.sync.dma_start(out=qf_sb, in_=qf_d[:])
            wcat = consts.tile([P, 2, DCAT], F32)
            nc.sync.dma_start(out=wcat[:, 0, 0:IDIM], in_=wi_d[0:P, :])
            nc.sync.dma_start(out=wcat[:, 1, 0:IDIM], in_=wi_d[P : 2 * P, :])
            nc.sync.dma_start(out=wcat[:, 0, IDIM:DCAT], in_=wm_d[0:P, :])
            nc.sync.dma_start(out=wcat[:, 1, IDIM:DCAT], in_=wm_d[P : 2 * P, :])
            bcat = consts.tile([NQ, DCAT], F32)
            nc.sync.dma_start(
                out=bcat[:, 0:IDIM],
                in_=bass.AP(tensor=bi_d, offset=0, ap=[[0, NQ], [1, IDIM]]),
            )
            nc.sync.dma_start(
                out=bcat[:, IDIM:DCAT],
                in_=bass.AP(tensor=bm_d, offset=0, ap=[[0, NQ], [1, MDIM]]),
            )

            qfT_sb = consts.tile([P, 2, NQ], F32)
            q48_sb = consts.tile([NQ, DCAT], F32)
            sfix = consts.tile([P, 2, DCAT + 1], F32)

            with tc.tile_pool(name="setup_ps", bufs=1, space=MemorySpace.PSUM) as sps:
                qfT_ps = sps.tile([P, 2, NQ], F32)
                for cc in range(2):
                    nc.tensor.transpose(
                        qfT_ps[:, cc, :],
                        qf_sb[:, cc * P : (cc + 1) * P],
                        ident[0:NQ, 0:NQ],
                    )
                nc.scalar.copy(qfT_sb, qfT_ps)
                q48_ps = sps.tile([NQ, DCAT], F32)
                nc.tensor.matmul(
                    q48_ps, qfT_sb[:, 0, :], wcat[:, 0, :], start=True, stop=False
                )
                nc.tensor.matmul(
                    q48_ps, qfT_sb[:, 1, :], wcat[:, 1, :], start=False, stop=True
                )
                nc.vector.tensor_add(q48_sb, q48_ps, bcat)

            # Sfix block-diagonal: [(j2,c), (j2,d)] = Qext[c,d]; Qext=[Q48|1]
            nc.vector.memset(sfix, 0.0)
            nc.vector.memset(sfix[0:NQ, 0, DCAT : DCAT + 1], 1.0)
            nc.vector.memset(sfix[NQ : 2 * NQ, 1, DCAT : DCAT + 1], 1.0)
            nc.vector.tensor_copy(sfix[0:NQ, 0, 0:DCAT], q48_sb)
            # partition-shifted copy must go through DMA
            nc.sync.dma_start(out=sfix[NQ : 2 * NQ, 1, 0:DCAT], in_=q48_sb)
            # compute-dtype copy for the pooling matmul: bf16 (FWL weight
            # load) or fp32r (single-pass fp32; plain f32 is 2-pass LO/HI)
            cdt = BF16 if USE_BF16 else mybir.dt.float32r
            sfix_r = consts.tile([P, 2, DCAT + 1], cdt)
            nc.vector.tensor_copy(sfix_r, sfix)

            # ---- main loop ----
            with (
                tc.tile_pool(name="ap_pool", bufs=3) as ap_pool,
                tc.tile_pool(name="ia_pool", bufs=3) as ia_pool,
                tc.tile_pool(name="mc_pool", bufs=3) as mc_pool,
                tc.tile_pool(name="out_pool", bufs=2) as out_pool,
                tc.tile_pool(name="gate_pool", bufs=2) as gate_pool,
                tc.tile_pool(name="aptr_pool", bufs=4) as aptr_pool,
                tc.tile_pool(name="small_pool", bufs=6) as small_pool,
                tc.tile_pool(name="scaled_pool", bufs=4) as scaled_pool,
                tc.tile_pool(name="psT", bufs=2, space=MemorySpace.PSUM) as psT,
                tc.tile_pool(name="psG", bufs=2, space=MemorySpace.PSUM) as psG,
            ):
                def emit_loads(it):
                    j0 = it * jt
                    # fg is host-repacked c-contiguous: full-rate DMA and
                    # each 2-j unit is one contiguous 128-col slice (matmul
                    # stationary operand requires a single free dimension)
                    fg_t = ap_pool.tile([P, jt, NQ], fg_dt, tag="fg")
                    nc.sync.dma_start(out=fg_t, in_=fgv[:, j0 : j0 + jt, :])
                    bg_t = ap_pool.tile([P, jt], F32, tag="bg")
                    nc.sync.dma_start(out=bg_t, in_=bgv[:, j0 : j0 + jt])
                    ia_t = ia_pool.tile([P, jt, IDIM], F32, tag="ia")
                    nc.sync.dma_start(out=ia_t, in_=iav[:, j0 : j0 + jt, :])
                    mc_t = mc_pool.tile([P, jt, MDIM], F32, tag="mc")
                    nc.sync.dma_start(out=mc_t, in_=mcv[:, j0 : j0 + jt, :])
                    return fg_t, bg_t, ia_t, mc_t

                loads_q = [emit_loads(0)]
                if ntiles > 1:
                    loads_q.append(emit_loads(1))
                for it in range(ntiles):
                    j0 = it * jt
                    fg_t, bg_t, ia_t, mc_t = loads_q.pop(0)
                    # prefetch loads two tiles ahead: slots (bufs=3) are
                    # long-free so the DMAs issue and transfer early
                    if it + 2 < ntiles:
                        loads_q.append(emit_loads(it + 2))

                    # gate = 1 - bg  (bg = prob column 0 of each j)
                    gate_t = gate_pool.tile([P, jt], F32)
                    nc.vector.tensor_scalar(
                        out=gate_t,
                        in0=bg_t,
                        scalar1=-1.0,
                        scalar2=1.0,
                        op0=mybir.AluOpType.mult,
                        op1=mybir.AluOpType.add,
                    )

                    oi_t = out_pool.tile([P, jt, IDIM], F32, tag="oi")
                    om_t = out_pool.tile([P, jt, MDIM], F32, tag="om")

                    # One group = gj j-columns = nunit 2-j transposes packed
                    # into ONE psum bank + ONE ACT copy.  Transposes for the
                    # next groups are emitted before this pair's matmuls so
                    # the in-order PE never stalls on the ACT copy.
                    def t_group(g):
                        gl = g * gj
                        apT_ps = psT.tile([P, nunit, P], fg_dt, tag="apT_ps")
                        for u in range(nunit):
                            jl = gl + u * 2
                            nc.tensor.transpose(
                                apT_ps[:, u, :],
                                fg_t[:, jl : jl + 2, :].rearrange(
                                    "p j c -> p (j c)"
                                ),
                                ident_c,
                            )
                        # single psum->sbuf copy (casts to the compute dtype)
                        apT_sb = aptr_pool.tile([P, nunit, P], cdt, tag="apT_sb")
                        nc.scalar.copy(apT_sb, apT_ps)
                        return apT_sb

                    # epilogue processes PAIRS of groups from one 2-bank
                    # psum tile [P, 2, 512] to halve DVE op count
                    npair = ngrp // 2
                    BANKF = 512  # f32 elems per psum bank
                    apT_q = [t_group(0), t_group(1)]
                    for pr in range(npair):
                        if 2 * pr + 3 < ngrp:
                            apT_q.append(t_group(2 * pr + 2))
                            apT_q.append(t_group(2 * pr + 3))
                        pg2 = psG.tile([P, 2, BANKF], F32)
                        for h in range(2):
                            apT_h = apT_q.pop(0)
                            for u in range(nunit):
                                nc.tensor.matmul(
                                    pg2[:, h, u * 98 : u * 98 + 98],
                                    apT_h[:, u, :],
                                    sfix_r,
                                    start=True,
                                    stop=True,
                                )
                        gl = pr * 2 * gj  # local j base of the pair (16 j)
                        jj = 2 * gj
                        # [P, 2, gj, 49] view of the pair's pooled+norm cols
                        pgv = pg2[:, :, 0 : gj * (DCAT + 1)].rearrange(
                            "p h (j k) -> p h j k", k=DCAT + 1
                        )
                        recip_t = small_pool.tile([P, 2, gj], F32, tag="recip")
                        nc.vector.reciprocal(recip_t, pgv[:, :, :, DCAT])
                        scale_t = small_pool.tile([P, 2, gj], F32, tag="scale")
                        nc.vector.tensor_mul(
                            scale_t,
                            recip_t,
                            gate_t[:, gl : gl + jj].rearrange(
                                "p (h j) -> p h j", h=2
                            ),
                        )
                        scale_b = bass.AP(
                            tensor=scale_t.tensor,
                            offset=scale_t.offset,
                            ap=[
                                scale_t.ap[0],
                                scale_t.ap[1],
                                scale_t.ap[2],
                                [0, DCAT],
                            ],
                        )
                        scaled = scaled_pool.tile([P, 2, gj, DCAT], F32)
                        nc.vector.tensor_mul(
                            scaled, pgv[:, :, :, 0:DCAT], scale_b
                        )
                        # adds split across GpSimd (inst) and DVE (motion)
                        nc.gpsimd.tensor_add(
                            oi_t[:, gl : gl + jj, :].rearrange(
                                "p (h j) c -> p h j c", h=2
                            ),
                            scaled[:, :, :, 0:IDIM],
                            ia_t[:, gl : gl + jj, :].rearrange(
                                "p (h j) c -> p h j c", h=2
                            ),
                        )
                        nc.vector.tensor_add(
                            om_t[:, gl : gl + jj, :].rearrange(
                                "p (h j) c -> p h j c", h=2
                            ),
                            scaled[:, :, :, IDIM:DCAT],
                            mc_t[:, gl : gl + jj, :].rearrange(
                                "p (h j) c -> p h j c", h=2
                            ),
                        )

                    nc.sync.dma_start(out=oiv[:, j0 : j0 + jt, :], in_=oi_t)
                    nc.sync.dma_start(out=omv[:, j0 : j0 + jt, :], in_=om_t)

    if split:
        _split_multiwaits(nc)
    return nc


_NC_CACHE = {}


def _get_nc(npix=NPIX, jt=64, gj=8):
    key = (npix, jt, gj)
    if key not in _NC_CACHE:
        _NC_CACHE[key] = _build(npix, jt, gj)
    return _NC_CACHE[key]


def _make_in_maps(inputs):
    ap = np.ascontiguousarray(np.asarray(inputs["assignment_prob"], np.float32))
    ia = np.ascontiguousarray(np.asarray(inputs["instance_affinity"], np.float32))
    mc = np.ascontiguousarray(np.asarray(inputs["motion_code"], np.float32))
    qf = np.ascontiguousarray(np.asarray(inputs["query_feat"], np.float32))
    wi = np.ascontiguousarray(np.asarray(inputs["W_inst"], np.float32))
    bi = np.ascontiguousarray(np.asarray(inputs["b_inst"], np.float32))
    wm = np.ascontiguousarray(np.asarray(inputs["W_motion"], np.float32))
    bm = np.ascontiguousarray(np.asarray(inputs["b_motion"], np.float32))
    B, T = ap.shape[0], ap.shape[1]
    in_maps = []
    for b in range(B):
        for t in range(T):
            apc = ap[b, t].reshape(NPIX, NCOL)
            in_maps.append(
                {
                    "fg": (
                        np.ascontiguousarray(apc[:, 1:]).astype(
                            ml_dtypes.bfloat16
                        )
                        if USE_BF16
                        else np.ascontiguousarray(apc[:, 1:])
                    ),
                    "bg": np.ascontiguousarray(apc[:, 0]),
                    "ia": np.ascontiguousarray(ia[b, t].reshape(NPIX, IDIM)),
                    "mc": np.ascontiguousarray(mc[b, t].reshape(NPIX, MDIM)),
                    "qf": np.ascontiguousarray(qf[b, t]),
                    "wi": wi,
                    "bi": bi,
                    "wm": wm,
                    "bm": bm,
                }
            )
    return in_maps


def _run(inputs, trace=False, tmpdir=None):
    from concourse.bass_utils import run_bass_kernel_spmd

    nc = _get_nc()
    in_maps = _make_in_maps(inputs)
    res = run_bass_kernel_spmd(
        nc, in_maps, list(range(8)), trace=trace, tmpdir=tmpdir
    )
    ia = np.asarray(inputs["instance_affinity"])
    B, T, V, H, W_ = ia.shape[:5]
    inst = np.stack([res.results[k]["oi"] for k in range(8)]).reshape(
        B, T, V, H, W_, IDIM
    )
    mot = np.stack([res.results[k]["om"] for k in range(8)]).reshape(
        B, T, V, H, W_, MDIM
    )
    aq = np.asarray(inputs["assigned_query"])
    return (aq, inst, mot), res


def kernel(**inputs):
    out, _ = _run(inputs, trace=False)
    return out
